# revision 15
# baseline (speedup 1.0000x reference)
"""Trainium2 Bass kernel for nn_HA_unit (gnn_message_passing).

Math (per batch b, N = H*W spatial positions):
  wfeat = BN1(w1 @ x)                       [IC, N]   (BN folded on host)
  iw    = wfeat^T wfeat * IC^-0.5           [N, N]    symmetric
  nodes = node_w @ x + node_b               [N, IC]
  b0    = (sigmoid(iw) >= delta)            [N, N]    binary, symmetric
  bh_k  = b0^k  (k = 1, 2, 3)               exact integer counts
  hop_k = softmax(bh_k o iw) @ nodes        (k=2,3 are exact one-hot:
                                             min top-2 logit gap 2.2e3 / 1.2e6)
  out   = R1 @ x[:IC] + sum_i H_i @ hop_i^T + bias   (fuse/res/BN folded on host)

Sharding: 8 cores = 4 batches x 2 halves of N. Core (b, h) receives x[b]
with spatial positions rolled by h*N/2 so its rows are always 0..N/2-1.
All weights/x shipped f16; b0 fp8 (exact 0/1); bh2 f16 (counts <= 2154,
f16 exact to 2048, +-1 beyond -- logit budget ~10 vs gap 2250); bh3 bf16
(rel 2^-9 -> logit budget ~30k vs gap 1.2e6). End-to-end sim rel err 2.9e-4.
"""

import sys

sys.path.insert(0, "/opt/trn_rl_repo")

import numpy as np

P = 128
USE_GATHER = False


def _build(cin, ic, n, r, hop, thr):
    from concourse import bass, tile, bacc
    import concourse.mybir as mybir
    from concourse.masks import make_identity

    f32 = mybir.dt.float32
    f16 = mybir.dt.float16
    bf16 = mybir.dt.bfloat16
    fp8 = mybir.dt.float8e4
    AF = mybir.ActivationFunctionType
    ALU = mybir.AluOpType
    AX = mybir.AxisListType
    DR = mybir.MatmulPerfMode.DoubleRow

    ncin = cin // P          # 4  k-chunks over input channels
    nic = ic // P            # 2  chunks over inter channels
    nkn = n // P             # 32 k-chunks over N
    nrt = r // P             # 16 row tiles per core
    FB = 512
    nfb = n // FB            # 8
    ncout = cin // P         # 4

    nc = bacc.Bacc("TRN2", target_bir_lowering=False, debug=True)

    xb = nc.dram_tensor("xb", [cin, n], f16, kind="ExternalInput")
    iota_in = nc.dram_tensor("iota_in", [P, n], f32, kind="ExternalInput")
    w1T = nc.dram_tensor("w1T", [cin, ic], f16, kind="ExternalInput")
    nodeT = nc.dram_tensor("nodeT", [cin, ic], f16, kind="ExternalInput")
    nbrow = nc.dram_tensor("nbrow", [1, ic], f16, kind="ExternalInput")
    HT = nc.dram_tensor("HT", [hop * ic, cin], f16, kind="ExternalInput")
    R1T = nc.dram_tensor("R1T", [ic, cin], f16, kind="ExternalInput")
    biases = nc.dram_tensor("biases", [P, nic + ncout], f32, kind="ExternalInput")
    out = nc.dram_tensor("out", [cin, r], f32, kind="ExternalOutput")

    with tile.TileContext(nc) as tc:
        with (
            tc.tile_pool(name="dram", bufs=1, space="DRAM") as dpool,
            tc.tile_pool(name="consts", bufs=1) as consts,
        ):
            iwq = dpool.tile([r, n], f16, tag="iwq")
            bh2d = dpool.tile([r, n], f16, tag="bh2d")
            bh3d = dpool.tile([r, n], bf16, tag="bh3d")
            nodes_d = dpool.tile([n, ic], f16, tag="nodes_d")
            st_d = dpool.tile([r, n], f16, tag="st_d")

            identh = consts.tile([P, P], f16, tag="identh")
            make_identity(nc, identh[:])
            bias_sb = consts.tile([P, nic + ncout], f32, tag="bias_sb")
            nc.sync.dma_start(bias_sb[:], biases[:])
            ones1 = consts.tile([1, P], f16, tag="ones1")
            nc.vector.memset(ones1[:], 1.0)
            nbrow_sb = consts.tile([1, ic], f16, tag="nbrow_sb")
            nc.sync.dma_start(nbrow_sb[:], nbrow[:])
            nodes_sb = consts.tile([P, nkn, ic], f16, tag="nodes_sb")
            HT_sb = consts.tile([P, hop * nic, cin], f16, tag="HT_sb")
            nc.sync.dma_start(
                HT_sb[:], HT[:, :].rearrange("(k p) o -> p k o", p=P)
            )
            R1T_sb = consts.tile([P, nic, cin], f16, tag="R1T_sb")
            nc.sync.dma_start(
                R1T_sb[:], R1T[:, :].rearrange("(k p) o -> p k o", p=P)
            )

            with tc.tile_pool(name="b0top", bufs=1) as b0top_pool:
                b0t = b0top_pool.tile([P, nrt, n], fp8, tag="b0t")
                with tc.tile_pool(name="b0bot", bufs=1) as b0bot_pool:
                    b0b = b0bot_pool.tile([P, nkn - nrt, n], fp8, tag="b0b")

                    def b0_ap(k, sl):
                        if k < nrt:
                            return b0t[:, k, sl]
                        return b0b[:, k - nrt, sl]

                    def b0_ap2(k2, sl):
                        # pair of adjacent k-chunks for DoubleRow
                        if 2 * k2 < nrt:
                            return b0t[:, 2 * k2:2 * k2 + 2, sl]
                        return b0b[:, 2 * k2 - nrt:2 * k2 - nrt + 2, sl]

                    # ---------- Phase A: wfeat + nodes (stream x) ----------
                    with tc.tile_pool(name="wfp", bufs=1) as wfp:
                        wf_sb = wfp.tile([P, nic, n], f16, tag="wf_sb")
                        with (
                            tc.tile_pool(name="pa", bufs=1) as pa,
                            tc.tile_pool(name="pax", bufs=2) as pax,
                            tc.tile_pool(name="psA", bufs=2, space="PSUM") as psA,
                            tc.tile_pool(name="psN", bufs=2, space="PSUM") as psN,
                        ):
                            w1T_sb = pa.tile([P, ncin, ic], f16, tag="w1T_sb")
                            nc.sync.dma_start(
                                w1T_sb[:],
                                w1T[:, :].rearrange("(k p) o -> p k o", p=P),
                            )
                            nodeT_sb = pa.tile([P, ncin, ic], f16, tag="nodeT_sb")
                            nc.sync.dma_start(
                                nodeT_sb[:],
                                nodeT[:, :].rearrange("(k p) o -> p k o", p=P),
                            )
                            for fb in range(nfb):
                                x_blk = pax.tile([P, ncin, FB], f16, tag="x_blk")
                                nc.sync.dma_start(
                                    x_blk[:],
                                    xb[:, fb * FB:(fb + 1) * FB].rearrange(
                                        "(k p) n -> p k n", p=P
                                    ),
                                )
                                for oc in range(nic):
                                    ps = psA.tile([P, FB], f32, tag="psA")
                                    for k in range(ncin):
                                        nc.tensor.matmul(
                                            ps[:],
                                            w1T_sb[:, k, oc * P:(oc + 1) * P],
                                            x_blk[:, k, :],
                                            start=(k == 0),
                                            stop=(k == ncin - 1),
                                        )
                                    nc.scalar.activation(
                                        wf_sb[:, oc, fb * FB:(fb + 1) * FB],
                                        ps[:],
                                        AF.Identity,
                                        bias=bias_sb[:, oc:oc + 1],
                                    )
                                for sub in range(FB // P):
                                    ps = psN.tile([P, ic], f32, tag="psN")
                                    for k in range(ncin):
                                        nc.tensor.matmul(
                                            ps[:],
                                            x_blk[:, k, sub * P:(sub + 1) * P],
                                            nodeT_sb[:, k, :],
                                            start=(k == 0),
                                            stop=False,
                                        )
                                    nc.tensor.matmul(
                                        ps[:], ones1[:], nbrow_sb[:],
                                        start=False, stop=True,
                                    )
                                    nc.vector.tensor_copy(
                                        nodes_sb[:, fb * (FB // P) + sub, :], ps[:]
                                    )

                        nc.sync.dma_start(
                            nodes_d[:, :].rearrange("(t p) c -> p t c", p=P),
                            nodes_sb[:],
                        )

                        # ---------- Phase B: iw + b0 ----------
                        with (
                            tc.tile_pool(name="pb", bufs=2) as pb,
                            tc.tile_pool(name="psB", bufs=3, space="PSUM") as psB,
                        ):
                            for pc in range(nkn):
                                iw_row = pb.tile([P, n], f16, tag="iw_row")
                                for f in range(nfb):
                                    fsl = slice(f * FB, (f + 1) * FB)
                                    ps = psB.tile([P, FB], f32, tag="psB")
                                    for k in range(nic):
                                        nc.tensor.matmul(
                                            ps[:],
                                            wf_sb[:, k, pc * P:(pc + 1) * P],
                                            wf_sb[:, k, f * FB:(f + 1) * FB],
                                            start=(k == 0),
                                            stop=(k == nic - 1),
                                        )
                                    if f % 2 == 0:
                                        nc.scalar.activation(
                                            iw_row[:, fsl], ps[:], AF.Copy
                                        )
                                    else:
                                        nc.vector.tensor_copy(
                                            iw_row[:, fsl], ps[:]
                                        )
                                    nc.gpsimd.tensor_scalar(
                                        b0_ap(pc, fsl), iw_row[:, fsl],
                                        thr, None, op0=ALU.is_ge,
                                    )
                                if pc < nrt:
                                    nc.sync.dma_start(
                                        iwq[pc * P:(pc + 1) * P, :], iw_row[:]
                                    )

                    # ---------- Phase C+D: bh2 = b0^2, bh3 = b0^3 rows ----------
                    with (
                        tc.tile_pool(name="pcd", bufs=2) as pcd,
                        tc.tile_pool(name="psC", bufs=2, space="PSUM") as psC,
                        tc.tile_pool(name="psD", bufs=2, space="PSUM") as psD,
                    ):
                        for rt in range(nrt):
                            rsl = slice(rt * P, (rt + 1) * P)
                            bh2row = pcd.tile([P, n], f16, tag="bh2row")
                            for f in range(nfb):
                                fsl = slice(f * FB, (f + 1) * FB)
                                ps = psC.tile([P, FB], f32, tag="psC")
                                for k2 in range(nkn // 2):
                                    nc.tensor.matmul(
                                        ps[:],
                                        b0_ap2(k2, rsl),
                                        b0_ap2(k2, fsl),
                                        start=(k2 == 0),
                                        stop=(k2 == nkn // 2 - 1),
                                        perf_mode=DR,
                                    )
                                nc.scalar.activation(
                                    bh2row[:, fsl], ps[:], AF.Copy
                                )
                            nc.sync.dma_start(bh2d[rsl, :], bh2row[:])
                            bh2T = pcd.tile([P, nkn, P], f16, tag="bh2T")
                            nc.sync.dma_start(
                                bh2T[:], bh2d[rsl, :], transpose=True
                            )
                            bh3row = pcd.tile([P, n], bf16, tag="bh3row")
                            for f in range(nfb):
                                fsl = slice(f * FB, (f + 1) * FB)
                                ps = psD.tile([P, FB], f32, tag="psD")
                                for k in range(nkn):
                                    nc.tensor.matmul(
                                        ps[:],
                                        bh2T[:, k, :],
                                        b0_ap(k, fsl),
                                        start=(k == 0),
                                        stop=(k == nkn - 1),
                                    )
                                nc.scalar.activation(
                                    bh3row[:, fsl], ps[:], AF.Copy
                                )
                            nc.sync.dma_start(bh3d[rsl, :], bh3row[:])

                # ---------- Phase E: hops + fused output ----------
                with (
                    tc.tile_pool(name="pex", bufs=1) as pex,
                    tc.tile_pool(name="pe", bufs=2) as pe,
                    tc.tile_pool(name="pe1", bufs=1) as pe1,
                    tc.tile_pool(name="psT", bufs=2, space="PSUM") as psT,
                    tc.tile_pool(name="psE", bufs=2, space="PSUM") as psE,
                    tc.tile_pool(name="psO", bufs=2, space="PSUM") as psO,
                ):
                    iota_sb = pex.tile([P, n], f32, tag="iota_sb")
                    nc.sync.dma_start(iota_sb[:], iota_in[:])
                    for rt in range(nrt):
                        rsl = slice(rt * P, (rt + 1) * P)
                        iw_t = pe.tile([P, n], f16, tag="E_iw")
                        nc.sync.dma_start(iw_t[:], iwq[rsl, :])
                        bh2_t = pe1.tile([P, n], f16, tag="E_bh2")
                        nc.sync.dma_start(bh2_t[:], bh2d[rsl, :])
                        bh3_t = pe1.tile([P, n], bf16, tag="E_bh3")
                        nc.sync.dma_start(bh3_t[:], bh3d[rsl, :])
                        tTs = pe.tile([P, hop * nic, P], f16, tag="E_tTs")
                        xres_t = pe.tile([P, nic, P], f16, tag="E_xres")
                        nc.sync.dma_start(
                            xres_t[:],
                            xb[0:ic, rt * P:(rt + 1) * P].rearrange(
                                "(k p) q -> p k q", p=P
                            ),
                        )

                        # ---- hop 1: true softmax, matmul path ----
                        lt1 = pe1.tile([P, n], f16, tag="E_lt")
                        nc.vector.tensor_tensor(
                            lt1[:], iw_t[:], b0t[:, rt, :], op=ALU.mult
                        )
                        nmax = pe.tile([P, 1], f32, tag="E_nm")
                        nc.vector.tensor_reduce(
                            nmax[:], lt1[:], axis=AX.X, op=ALU.max, negate=True
                        )
                        pt = pe1.tile([P, n], f16, tag="E_pt")
                        zt = pe.tile([P, 1], f32, tag="E_z")
                        nc.scalar.activation(
                            pt[:], lt1[:], AF.Exp, bias=nmax[:], accum_out=zt[:]
                        )
                        rz = pe.tile([P, 1], f32, tag="E_rz")
                        nc.vector.reciprocal(rz[:], zt[:])
                        st = pe1.tile([P, n], f16, tag="E_st")
                        nc.scalar.activation(st[:], pt[:], AF.Copy, scale=rz[:])
                        sT = pe1.tile([P, nkn, P], f16, tag="E_sT")
                        for j in range(nkn):
                            pst = psT.tile([P, P], f16, tag="E_pst")
                            nc.tensor.transpose(
                                pst[:], st[:, j * P:(j + 1) * P], identh[:]
                            )
                            if j % 2 == 0:
                                nc.vector.tensor_copy(sT[:, j, :], pst[:])
                            else:
                                nc.scalar.activation(sT[:, j, :], pst[:], AF.Copy)
                        for c in range(nic):
                            ps = psE.tile([P, P], f32, tag="E_ps")
                            for j in range(nkn):
                                nc.tensor.matmul(
                                    ps[:],
                                    nodes_sb[:, j, c * P:(c + 1) * P],
                                    sT[:, j, :],
                                    start=(j == 0),
                                    stop=(j == nkn - 1),
                                )
                            nc.vector.tensor_copy(tTs[:, c, :], ps[:])

                        # ---- hops 2/3: exact one-hot ----
                        for i in (1, 2):
                            src = bh2_t if i == 1 else bh3_t
                            ltd = f16 if i == 1 else bf16
                            lt = pe1.tile([P, n], ltd, tag="E_lt")
                            nc.vector.tensor_tensor(
                                lt[:], iw_t[:], src[:], op=ALU.mult
                            )
                            mx = pe.tile([P, 1], f32, tag="E_mx")
                            nc.vector.tensor_reduce(
                                mx[:], lt[:], axis=AX.X, op=ALU.max,
                                negate=not USE_GATHER,
                            )
                            if USE_GATHER:
                                # argmax + row gather from nodes_d
                                mask = pe1.tile([P, n], f16, tag="E_pt")
                                nc.vector.tensor_scalar(
                                    mask[:], lt[:], mx[:, 0:1], None,
                                    op0=ALU.is_equal,
                                )
                                junk = pe1.tile([P, n], f32, tag="E_junk")
                                idxf = pe.tile([P, 1], f32, tag="E_idxf")
                                nc.vector.tensor_tensor_reduce(
                                    junk[:], mask[:], iota_sb[:], 1.0, 0.0,
                                    op0=ALU.mult, op1=ALU.max,
                                    accum_out=idxf[:],
                                )
                                idxi = pe.tile([P, 1], mybir.dt.int32,
                                               tag="E_idxi")
                                nc.vector.tensor_copy(idxi[:], idxf[:])
                                t_g = pe.tile([P, ic], f16, tag="E_tg")
                                nc.gpsimd.indirect_dma_start(
                                    out=t_g[:],
                                    out_offset=None,
                                    in_=nodes_d[:],
                                    in_offset=bass.IndirectOffsetOnAxis(
                                        ap=idxi[:, 0:1], axis=0
                                    ),
                                )
                                for c in range(nic):
                                    pst = psT.tile([P, P], f16, tag="E_pst")
                                    nc.tensor.transpose(
                                        pst[:], t_g[:, c * P:(c + 1) * P],
                                        identh[:],
                                    )
                                    nc.vector.tensor_copy(
                                        tTs[:, i * nic + c, :], pst[:]
                                    )
                            else:
                                # exp(l - max) is exactly the selection mask
                                mask = pe1.tile([P, n], f16, tag="E_pt")
                                nc.scalar.activation(
                                    mask[:], lt[:], AF.Exp, bias=mx[:]
                                )
                                sT2 = pe1.tile([P, nkn, P], f16, tag="E_sT")
                                for j in range(nkn):
                                    pst = psT.tile([P, P], f16, tag="E_pst")
                                    nc.tensor.transpose(
                                        pst[:], mask[:, j * P:(j + 1) * P],
                                        identh[:],
                                    )
                                    if j % 2 == 0:
                                        nc.vector.tensor_copy(
                                            sT2[:, j, :], pst[:]
                                        )
                                    else:
                                        nc.scalar.activation(
                                            sT2[:, j, :], pst[:], AF.Copy
                                        )
                                for c in range(nic):
                                    ps = psE.tile([P, P], f32, tag="E_ps")
                                    for j in range(nkn):
                                        nc.tensor.matmul(
                                            ps[:],
                                            nodes_sb[:, j, c * P:(c + 1) * P],
                                            sT2[:, j, :],
                                            start=(j == 0),
                                            stop=(j == nkn - 1),
                                        )
                                    nc.vector.tensor_copy(
                                        tTs[:, i * nic + c, :], ps[:]
                                    )
                        out_t = pe.tile([P, ncout, P], f32, tag="E_out")
                        for oc in range(ncout):
                            osl = slice(oc * P, (oc + 1) * P)
                            ps = psO.tile([P, P], f32, tag="E_po")
                            for c in range(nic):
                                nc.tensor.matmul(
                                    ps[:],
                                    R1T_sb[:, c, osl],
                                    xres_t[:, c, :],
                                    start=(c == 0),
                                    stop=False,
                                )
                            for ii in range(hop * nic):
                                nc.tensor.matmul(
                                    ps[:],
                                    HT_sb[:, ii, osl],
                                    tTs[:, ii, :],
                                    start=False,
                                    stop=(ii == hop * nic - 1),
                                )
                            nc.scalar.activation(
                                out_t[:, oc, :], ps[:], AF.Identity,
                                bias=bias_sb[:, nic + oc:nic + oc + 1],
                            )
                        nc.sync.dma_start(
                            out[:, rsl].rearrange("(o p) q -> p o q", p=P),
                            out_t[:],
                        )

    nc.compile()
    return nc


def _host_prep(inputs, cin, ic, n, r, hop, eps):
    """Fold BN + fuse/res convs into weights; build per-core input maps."""

    def F(a):
        return np.asarray(a, dtype=np.float64)

    x = np.asarray(inputs["x"], dtype=np.float32)
    B = x.shape[0]
    xf = x.reshape(B, cin, n)

    s4 = float(ic) ** -0.25
    inv1 = 1.0 / np.sqrt(F(inputs["bn1_v"]) + eps) * F(inputs["bn1_g"])
    w1_eff = (inv1[:, None] * F(inputs["w1_w"])) * s4
    b1_eff = (F(inputs["w1_b"]) * inv1 + F(inputs["bn1_b"])
              - F(inputs["bn1_m"]) * inv1) * s4

    invf = 1.0 / np.sqrt(F(inputs["bnf_v"]) + eps) * F(inputs["bnf_g"])
    fuse_eff = invf[:, None] * F(inputs["fuse_w"])
    fuse_b_eff = (F(inputs["fuse_b"]) * invf + F(inputs["bnf_b"])
                  - F(inputs["bnf_m"]) * invf)

    invr = 1.0 / np.sqrt(F(inputs["bnr_v"]) + eps) * F(inputs["bnr_g"])
    res_eff = invr[:, None] * F(inputs["res_w"])
    res_b_eff = (F(inputs["res_b"]) * invr + F(inputs["bnr_b"])
                 - F(inputs["bnr_m"]) * invr)
    R1 = res_eff[:, :ic]
    R2 = res_eff[:, ic:]

    hop_w = F(inputs["hop_w"])
    hop_b = F(inputs["hop_b"])
    H_i = [R2 @ fuse_eff[:, i * ic:(i + 1) * ic] @ hop_w[i] for i in range(hop)]
    bias_out = res_b_eff + R2 @ (
        sum(fuse_eff[:, i * ic:(i + 1) * ic] @ hop_b[i] for i in range(hop))
        + fuse_b_eff
    )

    delta = float(np.asarray(inputs["delta"]).reshape(-1)[0])
    if delta <= 0.0:
        thr = -3.0e38
    elif delta >= 1.0:
        thr = 3.0e38
    else:
        thr = float(np.log(delta / (1.0 - delta)))

    nic = ic // P
    ncout = cin // P
    bias_pack = np.zeros((P, nic + ncout), np.float32)
    for oc in range(nic):
        bias_pack[:, oc] = b1_eff[oc * P:(oc + 1) * P]
    for oc in range(ncout):
        bias_pack[:, nic + oc] = bias_out[oc * P:(oc + 1) * P]

    f16c = lambda a: np.ascontiguousarray(a, dtype=np.float16)
    HT = np.concatenate([H_i[i].T for i in range(hop)], axis=0)  # [hop*ic, cin]
    iota = np.tile(np.arange(n, dtype=np.float32), (P, 1))
    shared = {
        "iota_in": iota,
        "w1T": f16c(w1_eff.T),
        "nodeT": f16c(F(inputs["node_w"]).T),
        "nbrow": f16c(F(inputs["node_b"]).reshape(1, ic)),
        "HT": f16c(HT),
        "R1T": f16c(R1.T),
        "biases": bias_pack,
    }

    n_cores = (B * n) // r
    halves = n // r
    in_maps = []
    for c in range(n_cores):
        b, h = c // halves, c % halves
        perm = (np.arange(n) + h * r) % n
        m = dict(shared)
        m["xb"] = f16c(xf[b][:, perm])
        in_maps.append(m)
    return in_maps, thr


_BUILD_CACHE = {}


def kernel(**inputs):
    from concourse import bass_utils

    cin, ic, hop, eps = 512, 256, 3, 1e-5
    x = np.asarray(inputs["x"])
    B, _, H, W = x.shape
    n = H * W
    n_cores = 8
    r = (B * n) // n_cores
    halves = n // r

    in_maps, thr = _host_prep(inputs, cin, ic, n, r, hop, eps)

    key = (cin, ic, n, r, hop, thr)
    if key not in _BUILD_CACHE:
        _BUILD_CACHE[key] = _build(cin, ic, n, r, hop, thr)
    nc = _BUILD_CACHE[key]

    res = bass_utils.run_bass_kernel_spmd(nc, in_maps, core_ids=list(range(n_cores)))

    out = np.empty((B, cin, n), np.float32)
    for c in range(n_cores):
        b, h = c // halves, c % halves
        out[b][:, h * r:(h + 1) * r] = res.results[c]["out"]
    return out.reshape(B, cin, H, W).astype(x.dtype)


# revision 16
# speedup vs baseline: 1.7115x; 1.7115x over previous
"""Trainium2 Bass kernel for nn_HA_unit (gnn_message_passing).

Math (per batch b, N = H*W spatial positions):
  wfeat = BN1(w1 @ x)                       [IC, N]   (BN folded on host)
  iw    = wfeat^T wfeat * IC^-0.5           [N, N]    symmetric
  nodes = node_w @ x + node_b               [N, IC]
  b0    = (sigmoid(iw) >= delta)            [N, N]    binary, symmetric
  bh_k  = b0^k  (k = 1, 2, 3)               exact integer counts
  hop_k = softmax(bh_k o iw) @ nodes        (k=2,3 are exact one-hot:
                                             min top-2 logit gap 2.2e3 / 1.2e6)
  out   = R1 @ x[:IC] + sum_i H_i @ hop_i^T + bias   (fuse/res/BN folded on host)

Sharding: 8 cores = 4 batches x 2 halves of N. Core (b, h) receives x[b]
with spatial positions rolled by h*N/2 so its rows are always 0..N/2-1.
All weights/x shipped f16; b0 fp8 (exact 0/1); bh2 f16 (counts <= 2154,
f16 exact to 2048, +-1 beyond -- logit budget ~10 vs gap 2250); bh3 bf16
(rel 2^-9 -> logit budget ~30k vs gap 1.2e6). End-to-end sim rel err 2.9e-4.
"""

import sys

sys.path.insert(0, "/opt/trn_rl_repo")

import numpy as np

P = 128
USE_GATHER = False


def _build(cin, ic, n, r, hop, thr):
    from concourse import bass, tile, bacc
    import concourse.mybir as mybir
    from concourse.masks import make_identity

    f32 = mybir.dt.float32
    f16 = mybir.dt.float16
    bf16 = mybir.dt.bfloat16
    fp8 = mybir.dt.float8e4
    AF = mybir.ActivationFunctionType
    ALU = mybir.AluOpType
    AX = mybir.AxisListType
    DR = mybir.MatmulPerfMode.DoubleRow

    ncin = cin // P          # 4  k-chunks over input channels
    nic = ic // P            # 2  chunks over inter channels
    nkn = n // P             # 32 k-chunks over N
    nrt = r // P             # 16 row tiles per core
    FB = 512
    nfb = n // FB            # 8
    ncout = cin // P         # 4

    nc = bacc.Bacc("TRN2", target_bir_lowering=False, debug=True)

    xb = nc.dram_tensor("xb", [cin, n], f16, kind="ExternalInput")
    iota_in = nc.dram_tensor("iota_in", [P, n], f32, kind="ExternalInput")
    w1T = nc.dram_tensor("w1T", [cin, ic], f16, kind="ExternalInput")
    nodeT = nc.dram_tensor("nodeT", [cin, ic], f16, kind="ExternalInput")
    nbrow = nc.dram_tensor("nbrow", [1, ic], f16, kind="ExternalInput")
    HT = nc.dram_tensor("HT", [hop * ic, cin], f16, kind="ExternalInput")
    R1T = nc.dram_tensor("R1T", [ic, cin], f16, kind="ExternalInput")
    biases = nc.dram_tensor("biases", [P, nic + ncout], f32, kind="ExternalInput")
    out = nc.dram_tensor("out", [cin, r], f32, kind="ExternalOutput")

    with tile.TileContext(nc) as tc:
        with (
            tc.tile_pool(name="dram", bufs=1, space="DRAM") as dpool,
            tc.tile_pool(name="consts", bufs=1) as consts,
        ):
            iwq = dpool.tile([r, n], f16, tag="iwq")
            bh2d = dpool.tile([r, n], f16, tag="bh2d")
            bh3d = dpool.tile([r, n], bf16, tag="bh3d")
            nodes_d = dpool.tile([n, ic], f16, tag="nodes_d")
            st_d = dpool.tile([r, n], f16, tag="st_d")

            identh = consts.tile([P, P], f16, tag="identh")
            make_identity(nc, identh[:])
            bias_sb = consts.tile([P, nic + ncout], f32, tag="bias_sb")
            nc.sync.dma_start(bias_sb[:], biases[:])
            ones1 = consts.tile([1, P], f16, tag="ones1")
            nc.vector.memset(ones1[:], 1.0)
            nbrow_sb = consts.tile([1, ic], f16, tag="nbrow_sb")
            nc.sync.dma_start(nbrow_sb[:], nbrow[:])
            nodes_sb = consts.tile([P, nkn, ic], f16, tag="nodes_sb")
            HT_sb = consts.tile([P, hop * nic, cin], f16, tag="HT_sb")
            nc.sync.dma_start(
                HT_sb[:], HT[:, :].rearrange("(k p) o -> p k o", p=P)
            )
            R1T_sb = consts.tile([P, nic, cin], f16, tag="R1T_sb")
            nc.sync.dma_start(
                R1T_sb[:], R1T[:, :].rearrange("(k p) o -> p k o", p=P)
            )

            with tc.tile_pool(name="b0top", bufs=1) as b0top_pool:
                b0t = b0top_pool.tile([P, nrt, n], fp8, tag="b0t")
                with tc.tile_pool(name="b0bot", bufs=1) as b0bot_pool:
                    b0b = b0bot_pool.tile([P, nkn - nrt, n], fp8, tag="b0b")

                    def b0_ap(k, sl):
                        if k < nrt:
                            return b0t[:, k, sl]
                        return b0b[:, k - nrt, sl]

                    def b0_ap2(k2, sl):
                        # pair of adjacent k-chunks for DoubleRow
                        if 2 * k2 < nrt:
                            return b0t[:, 2 * k2:2 * k2 + 2, sl]
                        return b0b[:, 2 * k2 - nrt:2 * k2 - nrt + 2, sl]

                    # ---------- Phase A: wfeat + nodes (stream x) ----------
                    with tc.tile_pool(name="wfp", bufs=1) as wfp:
                        wf_sb = wfp.tile([P, nic, n], f16, tag="wf_sb")
                        with (
                            tc.tile_pool(name="pa", bufs=1) as pa,
                            tc.tile_pool(name="pax", bufs=2) as pax,
                            tc.tile_pool(name="psA", bufs=2, space="PSUM") as psA,
                            tc.tile_pool(name="psN", bufs=2, space="PSUM") as psN,
                        ):
                            w1T_sb = pa.tile([P, ncin, ic], f16, tag="w1T_sb")
                            nc.sync.dma_start(
                                w1T_sb[:],
                                w1T[:, :].rearrange("(k p) o -> p k o", p=P),
                            )
                            nodeT_sb = pa.tile([P, ncin, ic], f16, tag="nodeT_sb")
                            nc.sync.dma_start(
                                nodeT_sb[:],
                                nodeT[:, :].rearrange("(k p) o -> p k o", p=P),
                            )
                            for fb in range(nfb):
                                x_blk = pax.tile([P, ncin, FB], f16, tag="x_blk")
                                nc.sync.dma_start(
                                    x_blk[:],
                                    xb[:, fb * FB:(fb + 1) * FB].rearrange(
                                        "(k p) n -> p k n", p=P
                                    ),
                                )
                                for oc in range(nic):
                                    ps = psA.tile([P, FB], f32, tag="psA")
                                    for k in range(ncin):
                                        nc.tensor.matmul(
                                            ps[:],
                                            w1T_sb[:, k, oc * P:(oc + 1) * P],
                                            x_blk[:, k, :],
                                            start=(k == 0),
                                            stop=(k == ncin - 1),
                                        )
                                    nc.scalar.activation(
                                        wf_sb[:, oc, fb * FB:(fb + 1) * FB],
                                        ps[:],
                                        AF.Identity,
                                        bias=bias_sb[:, oc:oc + 1],
                                    )
                                for sub in range(FB // P):
                                    ps = psN.tile([P, ic], f32, tag="psN")
                                    for k in range(ncin):
                                        nc.tensor.matmul(
                                            ps[:],
                                            x_blk[:, k, sub * P:(sub + 1) * P],
                                            nodeT_sb[:, k, :],
                                            start=(k == 0),
                                            stop=False,
                                        )
                                    nc.tensor.matmul(
                                        ps[:], ones1[:], nbrow_sb[:],
                                        start=False, stop=True,
                                    )
                                    nc.vector.tensor_copy(
                                        nodes_sb[:, fb * (FB // P) + sub, :], ps[:]
                                    )

                        nc.sync.dma_start(
                            nodes_d[:, :].rearrange("(t p) c -> p t c", p=P),
                            nodes_sb[:],
                        )

                        # ---------- Phase B: iw + b0 ----------
                        with (
                            tc.tile_pool(name="pb", bufs=2) as pb,
                            tc.tile_pool(name="psB", bufs=3, space="PSUM") as psB,
                        ):
                            for pc in range(nkn):
                                if pc < nrt:
                                    iw_row = pb.tile([P, n], f16, tag="iw_row")
                                else:
                                    iw_row = None
                                for f in range(nfb):
                                    ps = psB.tile([P, FB], f32, tag="psB")
                                    for k in range(nic):
                                        nc.tensor.matmul(
                                            ps[:],
                                            wf_sb[:, k, pc * P:(pc + 1) * P],
                                            wf_sb[:, k, f * FB:(f + 1) * FB],
                                            start=(k == 0),
                                            stop=(k == nic - 1),
                                        )
                                    nc.vector.tensor_scalar(
                                        b0_ap(pc, slice(f * FB, (f + 1) * FB)),
                                        ps[:], thr, None, op0=ALU.is_ge,
                                    )
                                    if pc < nrt:
                                        nc.scalar.activation(
                                            iw_row[:, f * FB:(f + 1) * FB],
                                            ps[:], AF.Copy,
                                        )
                                if pc < nrt:
                                    nc.sync.dma_start(
                                        iwq[pc * P:(pc + 1) * P, :], iw_row[:]
                                    )

                    # ---------- Phase C+D: bh2 = b0^2, bh3 = b0^3 rows ----------
                    with (
                        tc.tile_pool(name="pcd", bufs=2) as pcd,
                        tc.tile_pool(name="psC", bufs=2, space="PSUM") as psC,
                        tc.tile_pool(name="psD", bufs=2, space="PSUM") as psD,
                    ):
                        for rt in range(nrt):
                            rsl = slice(rt * P, (rt + 1) * P)
                            bh2row = pcd.tile([P, n], f16, tag="bh2row")
                            for f in range(nfb):
                                fsl = slice(f * FB, (f + 1) * FB)
                                ps = psC.tile([P, FB], f32, tag="psC")
                                for k2 in range(nkn // 2):
                                    nc.tensor.matmul(
                                        ps[:],
                                        b0_ap2(k2, rsl),
                                        b0_ap2(k2, fsl),
                                        start=(k2 == 0),
                                        stop=(k2 == nkn // 2 - 1),
                                        perf_mode=DR,
                                    )
                                nc.scalar.activation(
                                    bh2row[:, fsl], ps[:], AF.Copy
                                )
                            nc.sync.dma_start(bh2d[rsl, :], bh2row[:])
                            bh2T = pcd.tile([P, nkn, P], f16, tag="bh2T")
                            nc.sync.dma_start(
                                bh2T[:], bh2d[rsl, :], transpose=True
                            )
                            bh3row = pcd.tile([P, n], bf16, tag="bh3row")
                            for f in range(nfb):
                                fsl = slice(f * FB, (f + 1) * FB)
                                ps = psD.tile([P, FB], f32, tag="psD")
                                for k in range(nkn):
                                    nc.tensor.matmul(
                                        ps[:],
                                        bh2T[:, k, :],
                                        b0_ap(k, fsl),
                                        start=(k == 0),
                                        stop=(k == nkn - 1),
                                    )
                                nc.scalar.activation(
                                    bh3row[:, fsl], ps[:], AF.Copy
                                )
                            nc.sync.dma_start(bh3d[rsl, :], bh3row[:])

                # ---------- Phase E: hops + fused output ----------
                with (
                    tc.tile_pool(name="pex", bufs=1) as pex,
                    tc.tile_pool(name="pe", bufs=2) as pe,
                    tc.tile_pool(name="pe1", bufs=1) as pe1,
                    tc.tile_pool(name="psT", bufs=2, space="PSUM") as psT,
                    tc.tile_pool(name="psE", bufs=2, space="PSUM") as psE,
                    tc.tile_pool(name="psO", bufs=2, space="PSUM") as psO,
                ):
                    iota_sb = pex.tile([P, n], f32, tag="iota_sb")
                    nc.sync.dma_start(iota_sb[:], iota_in[:])
                    for rt in range(nrt):
                        rsl = slice(rt * P, (rt + 1) * P)
                        iw_t = pe.tile([P, n], f16, tag="E_iw")
                        nc.sync.dma_start(iw_t[:], iwq[rsl, :])
                        bh2_t = pe1.tile([P, n], f16, tag="E_bh2")
                        nc.sync.dma_start(bh2_t[:], bh2d[rsl, :])
                        bh3_t = pe1.tile([P, n], bf16, tag="E_bh3")
                        nc.sync.dma_start(bh3_t[:], bh3d[rsl, :])
                        tTs = pe.tile([P, hop * nic, P], f16, tag="E_tTs")
                        xres_t = pe.tile([P, nic, P], f16, tag="E_xres")
                        nc.sync.dma_start(
                            xres_t[:],
                            xb[0:ic, rt * P:(rt + 1) * P].rearrange(
                                "(k p) q -> p k q", p=P
                            ),
                        )

                        # ---- hop 1: true softmax, matmul path ----
                        lt1 = pe1.tile([P, n], f16, tag="E_lt")
                        nc.vector.tensor_tensor(
                            lt1[:], iw_t[:], b0t[:, rt, :], op=ALU.mult
                        )
                        nmax = pe.tile([P, 1], f32, tag="E_nm")
                        nc.vector.tensor_reduce(
                            nmax[:], lt1[:], axis=AX.X, op=ALU.max, negate=True
                        )
                        pt = pe1.tile([P, n], f16, tag="E_pt")
                        zt = pe.tile([P, 1], f32, tag="E_z")
                        nc.scalar.activation(
                            pt[:], lt1[:], AF.Exp, bias=nmax[:], accum_out=zt[:]
                        )
                        rz = pe.tile([P, 1], f32, tag="E_rz")
                        nc.vector.reciprocal(rz[:], zt[:])
                        st = pe1.tile([P, n], f16, tag="E_st")
                        nc.scalar.activation(st[:], pt[:], AF.Copy, scale=rz[:])
                        sT = pe1.tile([P, nkn, P], f16, tag="E_sT")
                        for j in range(nkn):
                            pst = psT.tile([P, P], f16, tag="E_pst")
                            nc.tensor.transpose(
                                pst[:], st[:, j * P:(j + 1) * P], identh[:]
                            )
                            if j % 2 == 0:
                                nc.vector.tensor_copy(sT[:, j, :], pst[:])
                            else:
                                nc.scalar.activation(sT[:, j, :], pst[:], AF.Copy)
                        for c in range(nic):
                            ps = psE.tile([P, P], f32, tag="E_ps")
                            for j in range(nkn):
                                nc.tensor.matmul(
                                    ps[:],
                                    nodes_sb[:, j, c * P:(c + 1) * P],
                                    sT[:, j, :],
                                    start=(j == 0),
                                    stop=(j == nkn - 1),
                                )
                            nc.vector.tensor_copy(tTs[:, c, :], ps[:])

                        # ---- hops 2/3: exact one-hot ----
                        for i in (1, 2):
                            src = bh2_t if i == 1 else bh3_t
                            ltd = f16 if i == 1 else bf16
                            lt = pe1.tile([P, n], ltd, tag="E_lt")
                            nc.vector.tensor_tensor(
                                lt[:], iw_t[:], src[:], op=ALU.mult
                            )
                            mx = pe.tile([P, 1], f32, tag="E_mx")
                            nc.vector.tensor_reduce(
                                mx[:], lt[:], axis=AX.X, op=ALU.max,
                                negate=not USE_GATHER,
                            )
                            if USE_GATHER:
                                # argmax + row gather from nodes_d
                                mask = pe1.tile([P, n], f16, tag="E_pt")
                                nc.vector.tensor_scalar(
                                    mask[:], lt[:], mx[:, 0:1], None,
                                    op0=ALU.is_equal,
                                )
                                junk = pe1.tile([P, n], f32, tag="E_junk")
                                idxf = pe.tile([P, 1], f32, tag="E_idxf")
                                nc.vector.tensor_tensor_reduce(
                                    junk[:], mask[:], iota_sb[:], 1.0, 0.0,
                                    op0=ALU.mult, op1=ALU.max,
                                    accum_out=idxf[:],
                                )
                                idxi = pe.tile([P, 1], mybir.dt.int32,
                                               tag="E_idxi")
                                nc.vector.tensor_copy(idxi[:], idxf[:])
                                t_g = pe.tile([P, ic], f16, tag="E_tg")
                                nc.gpsimd.indirect_dma_start(
                                    out=t_g[:],
                                    out_offset=None,
                                    in_=nodes_d[:],
                                    in_offset=bass.IndirectOffsetOnAxis(
                                        ap=idxi[:, 0:1], axis=0
                                    ),
                                )
                                for c in range(nic):
                                    pst = psT.tile([P, P], f16, tag="E_pst")
                                    nc.tensor.transpose(
                                        pst[:], t_g[:, c * P:(c + 1) * P],
                                        identh[:],
                                    )
                                    nc.vector.tensor_copy(
                                        tTs[:, i * nic + c, :], pst[:]
                                    )
                            else:
                                # exp(l - max) is exactly the selection mask
                                mask = pe1.tile([P, n], f16, tag="E_pt")
                                nc.scalar.activation(
                                    mask[:], lt[:], AF.Exp, bias=mx[:]
                                )
                                sT2 = pe1.tile([P, nkn, P], f16, tag="E_sT")
                                for j in range(nkn):
                                    pst = psT.tile([P, P], f16, tag="E_pst")
                                    nc.tensor.transpose(
                                        pst[:], mask[:, j * P:(j + 1) * P],
                                        identh[:],
                                    )
                                    if j % 2 == 0:
                                        nc.vector.tensor_copy(
                                            sT2[:, j, :], pst[:]
                                        )
                                    else:
                                        nc.scalar.activation(
                                            sT2[:, j, :], pst[:], AF.Copy
                                        )
                                for c in range(nic):
                                    ps = psE.tile([P, P], f32, tag="E_ps")
                                    for j in range(nkn):
                                        nc.tensor.matmul(
                                            ps[:],
                                            nodes_sb[:, j, c * P:(c + 1) * P],
                                            sT2[:, j, :],
                                            start=(j == 0),
                                            stop=(j == nkn - 1),
                                        )
                                    nc.vector.tensor_copy(
                                        tTs[:, i * nic + c, :], ps[:]
                                    )
                        out_t = pe.tile([P, ncout, P], f32, tag="E_out")
                        for oc in range(ncout):
                            osl = slice(oc * P, (oc + 1) * P)
                            ps = psO.tile([P, P], f32, tag="E_po")
                            for c in range(nic):
                                nc.tensor.matmul(
                                    ps[:],
                                    R1T_sb[:, c, osl],
                                    xres_t[:, c, :],
                                    start=(c == 0),
                                    stop=False,
                                )
                            for ii in range(hop * nic):
                                nc.tensor.matmul(
                                    ps[:],
                                    HT_sb[:, ii, osl],
                                    tTs[:, ii, :],
                                    start=False,
                                    stop=(ii == hop * nic - 1),
                                )
                            nc.scalar.activation(
                                out_t[:, oc, :], ps[:], AF.Identity,
                                bias=bias_sb[:, nic + oc:nic + oc + 1],
                            )
                        nc.sync.dma_start(
                            out[:, rsl].rearrange("(o p) q -> p o q", p=P),
                            out_t[:],
                        )

    nc.compile()
    return nc


def _host_prep(inputs, cin, ic, n, r, hop, eps):
    """Fold BN + fuse/res convs into weights; build per-core input maps."""

    def F(a):
        return np.asarray(a, dtype=np.float64)

    x = np.asarray(inputs["x"], dtype=np.float32)
    B = x.shape[0]
    xf = x.reshape(B, cin, n)

    s4 = float(ic) ** -0.25
    inv1 = 1.0 / np.sqrt(F(inputs["bn1_v"]) + eps) * F(inputs["bn1_g"])
    w1_eff = (inv1[:, None] * F(inputs["w1_w"])) * s4
    b1_eff = (F(inputs["w1_b"]) * inv1 + F(inputs["bn1_b"])
              - F(inputs["bn1_m"]) * inv1) * s4

    invf = 1.0 / np.sqrt(F(inputs["bnf_v"]) + eps) * F(inputs["bnf_g"])
    fuse_eff = invf[:, None] * F(inputs["fuse_w"])
    fuse_b_eff = (F(inputs["fuse_b"]) * invf + F(inputs["bnf_b"])
                  - F(inputs["bnf_m"]) * invf)

    invr = 1.0 / np.sqrt(F(inputs["bnr_v"]) + eps) * F(inputs["bnr_g"])
    res_eff = invr[:, None] * F(inputs["res_w"])
    res_b_eff = (F(inputs["res_b"]) * invr + F(inputs["bnr_b"])
                 - F(inputs["bnr_m"]) * invr)
    R1 = res_eff[:, :ic]
    R2 = res_eff[:, ic:]

    hop_w = F(inputs["hop_w"])
    hop_b = F(inputs["hop_b"])
    H_i = [R2 @ fuse_eff[:, i * ic:(i + 1) * ic] @ hop_w[i] for i in range(hop)]
    bias_out = res_b_eff + R2 @ (
        sum(fuse_eff[:, i * ic:(i + 1) * ic] @ hop_b[i] for i in range(hop))
        + fuse_b_eff
    )

    delta = float(np.asarray(inputs["delta"]).reshape(-1)[0])
    if delta <= 0.0:
        thr = -3.0e38
    elif delta >= 1.0:
        thr = 3.0e38
    else:
        thr = float(np.log(delta / (1.0 - delta)))

    nic = ic // P
    ncout = cin // P
    bias_pack = np.zeros((P, nic + ncout), np.float32)
    for oc in range(nic):
        bias_pack[:, oc] = b1_eff[oc * P:(oc + 1) * P]
    for oc in range(ncout):
        bias_pack[:, nic + oc] = bias_out[oc * P:(oc + 1) * P]

    f16c = lambda a: np.ascontiguousarray(a, dtype=np.float16)
    HT = np.concatenate([H_i[i].T for i in range(hop)], axis=0)  # [hop*ic, cin]
    iota = np.tile(np.arange(n, dtype=np.float32), (P, 1))
    shared = {
        "iota_in": iota,
        "w1T": f16c(w1_eff.T),
        "nodeT": f16c(F(inputs["node_w"]).T),
        "nbrow": f16c(F(inputs["node_b"]).reshape(1, ic)),
        "HT": f16c(HT),
        "R1T": f16c(R1.T),
        "biases": bias_pack,
    }

    n_cores = (B * n) // r
    halves = n // r
    in_maps = []
    for c in range(n_cores):
        b, h = c // halves, c % halves
        perm = (np.arange(n) + h * r) % n
        m = dict(shared)
        m["xb"] = f16c(xf[b][:, perm])
        in_maps.append(m)
    return in_maps, thr


_BUILD_CACHE = {}


def kernel(**inputs):
    from concourse import bass_utils

    cin, ic, hop, eps = 512, 256, 3, 1e-5
    x = np.asarray(inputs["x"])
    B, _, H, W = x.shape
    n = H * W
    n_cores = 8
    r = (B * n) // n_cores
    halves = n // r

    in_maps, thr = _host_prep(inputs, cin, ic, n, r, hop, eps)

    key = (cin, ic, n, r, hop, thr)
    if key not in _BUILD_CACHE:
        _BUILD_CACHE[key] = _build(cin, ic, n, r, hop, thr)
    nc = _BUILD_CACHE[key]

    res = bass_utils.run_bass_kernel_spmd(nc, in_maps, core_ids=list(range(n_cores)))

    out = np.empty((B, cin, n), np.float32)
    for c in range(n_cores):
        b, h = c // halves, c % halves
        out[b][:, h * r:(h + 1) * r] = res.results[c]["out"]
    return out.reshape(B, cin, H, W).astype(x.dtype)


# revision 22
# speedup vs baseline: 2.5268x; 1.4764x over previous
"""Trainium2 Bass kernel for nn_HA_unit (gnn_message_passing).

Math (per batch b, N = H*W spatial positions):
  wfeat = BN1(w1 @ x)                       [IC, N]   (BN folded on host)
  iw    = wfeat^T wfeat * IC^-0.5           [N, N]    symmetric
  nodes = node_w @ x + node_b               [N, IC]
  b0    = (sigmoid(iw) >= delta)            [N, N]    binary, symmetric
  bh_k  = b0^k  (k = 1, 2, 3)               exact integer counts
  hop_k = softmax(bh_k o iw) @ nodes        (k=2,3 are exact one-hot:
                                             min top-2 logit gap 2.2e3 / 1.2e6)
  out   = R1 @ x[:IC] + sum_i H_i @ hop_i^T + bias   (fuse/res/BN folded on host)

Sharding: 8 cores = 4 batches x 2 halves of N. Core (b, h) receives x[b]
with spatial positions rolled by h*N/2 so its rows are always 0..N/2-1.
All weights/x shipped f16; b0 fp8 (exact 0/1); bh2 f16 (counts <= 2154,
f16 exact to 2048, +-1 beyond -- logit budget ~10 vs gap 2250); bh3 bf16
(rel 2^-9 -> logit budget ~30k vs gap 1.2e6). End-to-end sim rel err 2.9e-4.
"""

import sys

sys.path.insert(0, "/opt/trn_rl_repo")

import numpy as np

P = 128
USE_GATHER = False


def _build(cin, ic, n, r, hop, thr):
    from concourse import bass, tile, bacc
    import concourse.mybir as mybir
    from concourse.masks import make_identity

    f32 = mybir.dt.float32
    f16 = mybir.dt.float16
    bf16 = mybir.dt.bfloat16
    fp8 = mybir.dt.float8e4
    AF = mybir.ActivationFunctionType
    ALU = mybir.AluOpType
    AX = mybir.AxisListType
    DR = mybir.MatmulPerfMode.DoubleRow

    ncin = cin // P          # 4  k-chunks over input channels
    nic = ic // P            # 2  chunks over inter channels
    nkn = n // P             # 32 k-chunks over N
    nrt = r // P             # 16 row tiles per core
    FB = 512
    nfb = n // FB            # 8
    ncout = cin // P         # 4

    nc = bacc.Bacc("TRN2", target_bir_lowering=False, debug=True)

    xb = nc.dram_tensor("xb", [cin, n], f16, kind="ExternalInput")
    iota_in = nc.dram_tensor("iota_in", [P, n], f32, kind="ExternalInput")
    w1T = nc.dram_tensor("w1T", [cin, ic], f16, kind="ExternalInput")
    nodeT = nc.dram_tensor("nodeT", [cin, ic], f16, kind="ExternalInput")
    nbrow = nc.dram_tensor("nbrow", [1, ic], f16, kind="ExternalInput")
    HT = nc.dram_tensor("HT", [hop * ic, cin], f16, kind="ExternalInput")
    R1T = nc.dram_tensor("R1T", [ic, cin], f16, kind="ExternalInput")
    biases = nc.dram_tensor("biases", [P, nic + ncout], f32, kind="ExternalInput")
    out = nc.dram_tensor("out", [cin, r], f32, kind="ExternalOutput")

    with tile.TileContext(nc) as tc:
        with (
            tc.tile_pool(name="dram", bufs=1, space="DRAM") as dpool,
            tc.tile_pool(name="consts", bufs=1) as consts,
        ):
            iwq = dpool.tile([r, n], f16, tag="iwq")
            bh2d = dpool.tile([r, n], f16, tag="bh2d")
            nodes_d = dpool.tile([n, ic], f16, tag="nodes_d")
            st1_d = dpool.tile([r, n], f16, tag="st1_d")
            st2_d = dpool.tile([r, n], f16, tag="st2_d")
            st3_d = dpool.tile([r, n], f16, tag="st3_d")
            nodes_d = dpool.tile([n, ic], f16, tag="nodes_d")
            st_d = dpool.tile([r, n], f16, tag="st_d")

            identh = consts.tile([P, P], f16, tag="identh")
            make_identity(nc, identh[:])
            bias_sb = consts.tile([P, nic + ncout], f32, tag="bias_sb")
            nc.sync.dma_start(bias_sb[:], biases[:])
            ones1 = consts.tile([1, P], f16, tag="ones1")
            nc.vector.memset(ones1[:], 1.0)
            nbrow_sb = consts.tile([1, ic], f16, tag="nbrow_sb")
            nc.sync.dma_start(nbrow_sb[:], nbrow[:])

            with tc.tile_pool(name="b0top", bufs=1) as b0top_pool:
                b0t = b0top_pool.tile([P, nrt, n], fp8, tag="b0t")
                with tc.tile_pool(name="b0bot", bufs=1) as b0bot_pool:
                    b0b = b0bot_pool.tile([P, nkn - nrt, n], fp8, tag="b0b")

                    def b0_ap(k, sl):
                        if k < nrt:
                            return b0t[:, k, sl]
                        return b0b[:, k - nrt, sl]

                    def b0_ap2(k2, sl):
                        # pair of adjacent k-chunks for DoubleRow
                        if 2 * k2 < nrt:
                            return b0t[:, 2 * k2:2 * k2 + 2, sl]
                        return b0b[:, 2 * k2 - nrt:2 * k2 - nrt + 2, sl]

                    # ---------- Phase A: wfeat + nodes (stream x) ----------
                    with tc.tile_pool(name="wfp", bufs=1) as wfp:
                        wf_sb = wfp.tile([P, nic, n], f16, tag="wf_sb")
                        nodes_sb = wfp.tile([P, nkn, ic], f16, tag="nodes_sb")
                        with (
                            tc.tile_pool(name="pa", bufs=1) as pa,
                            tc.tile_pool(name="pax", bufs=2) as pax,
                            tc.tile_pool(name="psA", bufs=2, space="PSUM") as psA,
                            tc.tile_pool(name="psN", bufs=2, space="PSUM") as psN,
                        ):
                            w1T_sb = pa.tile([P, ncin, ic], f16, tag="w1T_sb")
                            nc.sync.dma_start(
                                w1T_sb[:],
                                w1T[:, :].rearrange("(k p) o -> p k o", p=P),
                            )
                            nodeT_sb = pa.tile([P, ncin, ic], f16, tag="nodeT_sb")
                            nc.sync.dma_start(
                                nodeT_sb[:],
                                nodeT[:, :].rearrange("(k p) o -> p k o", p=P),
                            )
                            for fb in range(nfb):
                                x_blk = pax.tile([P, ncin, FB], f16, tag="x_blk")
                                nc.sync.dma_start(
                                    x_blk[:],
                                    xb[:, fb * FB:(fb + 1) * FB].rearrange(
                                        "(k p) n -> p k n", p=P
                                    ),
                                )
                                for oc in range(nic):
                                    ps = psA.tile([P, FB], f32, tag="psA")
                                    for k in range(ncin):
                                        nc.tensor.matmul(
                                            ps[:],
                                            w1T_sb[:, k, oc * P:(oc + 1) * P],
                                            x_blk[:, k, :],
                                            start=(k == 0),
                                            stop=(k == ncin - 1),
                                        )
                                    nc.scalar.activation(
                                        wf_sb[:, oc, fb * FB:(fb + 1) * FB],
                                        ps[:],
                                        AF.Identity,
                                        bias=bias_sb[:, oc:oc + 1],
                                    )
                                for sub in range(FB // P):
                                    ps = psN.tile([P, ic], f32, tag="psN")
                                    for k in range(ncin):
                                        nc.tensor.matmul(
                                            ps[:],
                                            x_blk[:, k, sub * P:(sub + 1) * P],
                                            nodeT_sb[:, k, :],
                                            start=(k == 0),
                                            stop=False,
                                        )
                                    nc.tensor.matmul(
                                        ps[:], ones1[:], nbrow_sb[:],
                                        start=False, stop=True,
                                    )
                                    nc.vector.tensor_copy(
                                        nodes_sb[:, fb * (FB // P) + sub, :], ps[:]
                                    )

                        nc.sync.dma_start(
                            nodes_d[:, :].rearrange("(t p) c -> p t c", p=P),
                            nodes_sb[:],
                        )

                        nc.sync.dma_start(
                            nodes_d[:, :].rearrange("(t p) c -> p t c", p=P),
                            nodes_sb[:],
                        )

                        # ---------- Phase B: iw + b0 ----------
                        with (
                            tc.tile_pool(name="pb", bufs=4) as pb,
                            tc.tile_pool(name="psB", bufs=3, space="PSUM") as psB,
                        ):
                            # f-outer so b0 column-blocks finish early and
                            # phase C can start while B is still running
                            for f in range(nfb):
                                fsl = slice(f * FB, (f + 1) * FB)
                                for pc in range(nkn):
                                    ps = psB.tile([P, FB], f32, tag="psB")
                                    for k in range(nic):
                                        nc.tensor.matmul(
                                            ps[:],
                                            wf_sb[:, k, pc * P:(pc + 1) * P],
                                            wf_sb[:, k, fsl],
                                            start=(k == 0),
                                            stop=(k == nic - 1),
                                        )
                                    nc.vector.tensor_scalar(
                                        b0_ap(pc, fsl),
                                        ps[:], thr, None, op0=ALU.is_ge,
                                    )
                                    if pc < nrt:
                                        iw_ev = pb.tile(
                                            [P, FB], f16, tag="iw_ev"
                                        )
                                        nc.scalar.activation(
                                            iw_ev[:], ps[:], AF.Copy
                                        )
                                        nc.sync.dma_start(
                                            iwq[pc * P:(pc + 1) * P, fsl],
                                            iw_ev[:],
                                        )

                    # ---------- Phase C+D: bh2 = b0^2, bh3 = b0^3 rows ----------
                    with (
                        tc.tile_pool(name="pcd", bufs=1) as pcd,
                        tc.tile_pool(name="pcd8", bufs=1) as pcd8,
                        tc.tile_pool(name="smp", bufs=1) as smp,
                        tc.tile_pool(name="psC", bufs=2, space="PSUM") as psC,
                        tc.tile_pool(name="psD", bufs=2, space="PSUM") as psD,
                    ):
                        for rt in range(nrt):
                            rsl = slice(rt * P, (rt + 1) * P)
                            bh2row = pcd.tile([P, n], f16, tag="bh2row")
                            for f in range(nfb):
                                fsl = slice(f * FB, (f + 1) * FB)
                                ps = psC.tile([P, FB], f32, tag="psC")
                                for k2 in range(nkn // 2):
                                    nc.tensor.matmul(
                                        ps[:],
                                        b0_ap2(k2, rsl),
                                        b0_ap2(k2, fsl),
                                        start=(k2 == 0),
                                        stop=(k2 == nkn // 2 - 1),
                                        perf_mode=DR,
                                    )
                                nc.scalar.activation(
                                    bh2row[:, fsl], ps[:], AF.Copy
                                )
                            nc.sync.dma_start(bh2d[rsl, :], bh2row[:])
                            bh2T = pcd.tile([P, nkn, P], f16, tag="bh2T")
                            nc.sync.dma_start(
                                bh2T[:], bh2d[rsl, :], transpose=True
                            )
                            # bh3 only feeds the hop-3 argmax (min logit gap
                            # 1.2e6, scale-invariant): fp8(bh2/8) is exact
                            # enough (0 argmax flips) and enables DoubleRow
                            bh2T8 = pcd8.tile([P, nkn, P], fp8, tag="bh2T8")
                            nc.vector.tensor_scalar_mul(
                                bh2T8[:], bh2T[:], 0.0625
                            )
                            bh3row = pcd.tile([P, n], bf16, tag="bh3row")
                            for f in range(nfb):
                                fsl = slice(f * FB, (f + 1) * FB)
                                ps = psD.tile([P, FB], f32, tag="psD")
                                for k2 in range(nkn // 2):
                                    nc.tensor.matmul(
                                        ps[:],
                                        bh2T8[:, 2 * k2:2 * k2 + 2, :],
                                        b0_ap2(k2, fsl),
                                        start=(k2 == 0),
                                        stop=(k2 == nkn // 2 - 1),
                                        perf_mode=DR,
                                    )
                                nc.scalar.activation(
                                    bh3row[:, fsl], ps[:], AF.Copy
                                )
                            # softmax/one-hot for all 3 hops, hidden under
                            # the C/D matmuls; masks staged to DRAM
                            iw_t = smp.tile([P, n], f16, tag="sm_iw")
                            nc.sync.dma_start(iw_t[:], iwq[rsl, :])
                            lt = smp.tile([P, n], f16, tag="sm_a")
                            nc.vector.tensor_tensor(
                                lt[:], iw_t[:], b0t[:, rt, :], op=ALU.mult
                            )
                            nm1 = smp.tile([P, 1], f32, tag="sm_nm1")
                            nc.vector.tensor_reduce(
                                nm1[:], lt[:], axis=AX.X, op=ALU.max,
                                negate=True,
                            )
                            pt1 = smp.tile([P, n], f16, tag="sm_b")
                            zt = smp.tile([P, 1], f32, tag="sm_z")
                            nc.scalar.activation(
                                pt1[:], lt[:], AF.Exp, bias=nm1[:],
                                accum_out=zt[:],
                            )
                            rz = smp.tile([P, 1], f32, tag="sm_rz")
                            nc.vector.reciprocal(rz[:], zt[:])
                            st1 = smp.tile([P, n], f16, tag="sm_a")
                            nc.scalar.activation(
                                st1[:], pt1[:], AF.Copy, scale=rz[:]
                            )
                            nc.sync.dma_start(st1_d[rsl, :], st1[:])
                            lt2 = smp.tile([P, n], f16, tag="sm_b")
                            nc.vector.tensor_tensor(
                                lt2[:], iw_t[:], bh2row[:], op=ALU.mult
                            )
                            nm2 = smp.tile([P, 1], f32, tag="sm_nm2")
                            nc.vector.tensor_reduce(
                                nm2[:], lt2[:], axis=AX.X, op=ALU.max,
                                negate=True,
                            )
                            st2 = smp.tile([P, n], f16, tag="sm_a")
                            nc.scalar.activation(
                                st2[:], lt2[:], AF.Exp, bias=nm2[:]
                            )
                            nc.sync.dma_start(st2_d[rsl, :], st2[:])
                            lt3 = smp.tile([P, n], bf16, tag="sm_b")
                            nc.vector.tensor_tensor(
                                lt3[:], iw_t[:], bh3row[:], op=ALU.mult
                            )
                            nm3 = smp.tile([P, 1], f32, tag="sm_nm3")
                            nc.vector.tensor_reduce(
                                nm3[:], lt3[:], axis=AX.X, op=ALU.max,
                                negate=True,
                            )
                            st3 = smp.tile([P, n], f16, tag="sm_iw")
                            nc.scalar.activation(
                                st3[:], lt3[:], AF.Exp, bias=nm3[:]
                            )
                            nc.sync.dma_start(st3_d[rsl, :], st3[:])

            # ---------- Phase E: hop matmuls + fused output ----------
            with (
                tc.tile_pool(name="pex", bufs=1) as pex,
                tc.tile_pool(name="pe", bufs=2) as pe,
                tc.tile_pool(name="psE", bufs=2, space="PSUM") as psE,
                tc.tile_pool(name="psO", bufs=2, space="PSUM") as psO,
            ):
                nodes2 = pex.tile([P, nkn, ic], f16, tag="nodes2")
                nc.sync.dma_start(
                    nodes2[:], nodes_d[:, :].rearrange("(t p) c -> p t c", p=P)
                )
                HT_sb = pex.tile([P, hop * nic, cin], f16, tag="HT_sb")
                nc.sync.dma_start(
                    HT_sb[:], HT[:, :].rearrange("(k p) o -> p k o", p=P)
                )
                R1T_sb = pex.tile([P, nic, cin], f16, tag="R1T_sb")
                nc.sync.dma_start(
                    R1T_sb[:], R1T[:, :].rearrange("(k p) o -> p k o", p=P)
                )
                for rt in range(nrt):
                    rsl = slice(rt * P, (rt + 1) * P)
                    xres_t = pe.tile([P, nic, P], f16, tag="E_xres")
                    nc.sync.dma_start(
                        xres_t[:],
                        xb[0:ic, rt * P:(rt + 1) * P].rearrange(
                            "(k p) q -> p k q", p=P
                        ),
                    )
                    tTs = pe.tile([P, hop * nic, P], f16, tag="E_tTs")
                    for i, std in enumerate((st1_d, st2_d, st3_d)):
                        sT = pe.tile([P, nkn, P], f16, tag="E_sT")
                        nc.sync.dma_start(sT[:], std[rsl, :], transpose=True)
                        for c in range(nic):
                            ps = psE.tile([P, P], f32, tag="E_ps")
                            for j in range(nkn):
                                nc.tensor.matmul(
                                    ps[:],
                                    nodes2[:, j, c * P:(c + 1) * P],
                                    sT[:, j, :],
                                    start=(j == 0),
                                    stop=(j == nkn - 1),
                                )
                            nc.vector.tensor_copy(tTs[:, i * nic + c, :], ps[:])
                    out_t = pe.tile([P, ncout, P], f32, tag="E_out")
                    for oc in range(ncout):
                        osl = slice(oc * P, (oc + 1) * P)
                        ps = psO.tile([P, P], f32, tag="E_po")
                        for c in range(nic):
                            nc.tensor.matmul(
                                ps[:],
                                R1T_sb[:, c, osl],
                                xres_t[:, c, :],
                                start=(c == 0),
                                stop=False,
                            )
                        for ii in range(hop * nic):
                            nc.tensor.matmul(
                                ps[:],
                                HT_sb[:, ii, osl],
                                tTs[:, ii, :],
                                start=False,
                                stop=(ii == hop * nic - 1),
                            )
                        nc.scalar.activation(
                            out_t[:, oc, :], ps[:], AF.Identity,
                            bias=bias_sb[:, nic + oc:nic + oc + 1],
                        )
                    nc.sync.dma_start(
                        out[:, rsl].rearrange("(o p) q -> p o q", p=P),
                        out_t[:],
                    )

    nc.compile()
    return nc


def _host_prep(inputs, cin, ic, n, r, hop, eps):
    """Fold BN + fuse/res convs into weights; build per-core input maps."""

    def F(a):
        return np.asarray(a, dtype=np.float64)

    x = np.asarray(inputs["x"], dtype=np.float32)
    B = x.shape[0]
    xf = x.reshape(B, cin, n)

    s4 = float(ic) ** -0.25
    inv1 = 1.0 / np.sqrt(F(inputs["bn1_v"]) + eps) * F(inputs["bn1_g"])
    w1_eff = (inv1[:, None] * F(inputs["w1_w"])) * s4
    b1_eff = (F(inputs["w1_b"]) * inv1 + F(inputs["bn1_b"])
              - F(inputs["bn1_m"]) * inv1) * s4

    invf = 1.0 / np.sqrt(F(inputs["bnf_v"]) + eps) * F(inputs["bnf_g"])
    fuse_eff = invf[:, None] * F(inputs["fuse_w"])
    fuse_b_eff = (F(inputs["fuse_b"]) * invf + F(inputs["bnf_b"])
                  - F(inputs["bnf_m"]) * invf)

    invr = 1.0 / np.sqrt(F(inputs["bnr_v"]) + eps) * F(inputs["bnr_g"])
    res_eff = invr[:, None] * F(inputs["res_w"])
    res_b_eff = (F(inputs["res_b"]) * invr + F(inputs["bnr_b"])
                 - F(inputs["bnr_m"]) * invr)
    R1 = res_eff[:, :ic]
    R2 = res_eff[:, ic:]

    hop_w = F(inputs["hop_w"])
    hop_b = F(inputs["hop_b"])
    H_i = [R2 @ fuse_eff[:, i * ic:(i + 1) * ic] @ hop_w[i] for i in range(hop)]
    bias_out = res_b_eff + R2 @ (
        sum(fuse_eff[:, i * ic:(i + 1) * ic] @ hop_b[i] for i in range(hop))
        + fuse_b_eff
    )

    delta = float(np.asarray(inputs["delta"]).reshape(-1)[0])
    if delta <= 0.0:
        thr = -3.0e38
    elif delta >= 1.0:
        thr = 3.0e38
    else:
        thr = float(np.log(delta / (1.0 - delta)))

    nic = ic // P
    ncout = cin // P
    bias_pack = np.zeros((P, nic + ncout), np.float32)
    for oc in range(nic):
        bias_pack[:, oc] = b1_eff[oc * P:(oc + 1) * P]
    for oc in range(ncout):
        bias_pack[:, nic + oc] = bias_out[oc * P:(oc + 1) * P]

    f16c = lambda a: np.ascontiguousarray(a, dtype=np.float16)
    HT = np.concatenate([H_i[i].T for i in range(hop)], axis=0)  # [hop*ic, cin]
    iota = np.tile(np.arange(n, dtype=np.float32), (P, 1))
    shared = {
        "iota_in": iota,
        "w1T": f16c(w1_eff.T),
        "nodeT": f16c(F(inputs["node_w"]).T),
        "nbrow": f16c(F(inputs["node_b"]).reshape(1, ic)),
        "HT": f16c(HT),
        "R1T": f16c(R1.T),
        "biases": bias_pack,
    }

    n_cores = (B * n) // r
    halves = n // r
    in_maps = []
    for c in range(n_cores):
        b, h = c // halves, c % halves
        perm = (np.arange(n) + h * r) % n
        m = dict(shared)
        m["xb"] = f16c(xf[b][:, perm])
        in_maps.append(m)
    return in_maps, thr


_BUILD_CACHE = {}


def kernel(**inputs):
    from concourse import bass_utils

    cin, ic, hop, eps = 512, 256, 3, 1e-5
    x = np.asarray(inputs["x"])
    B, _, H, W = x.shape
    n = H * W
    n_cores = 8
    r = (B * n) // n_cores
    halves = n // r

    in_maps, thr = _host_prep(inputs, cin, ic, n, r, hop, eps)

    key = (cin, ic, n, r, hop, thr)
    if key not in _BUILD_CACHE:
        _BUILD_CACHE[key] = _build(cin, ic, n, r, hop, thr)
    nc = _BUILD_CACHE[key]

    res = bass_utils.run_bass_kernel_spmd(nc, in_maps, core_ids=list(range(n_cores)))

    out = np.empty((B, cin, n), np.float32)
    for c in range(n_cores):
        b, h = c // halves, c % halves
        out[b][:, h * r:(h + 1) * r] = res.results[c]["out"]
    return out.reshape(B, cin, H, W).astype(x.dtype)
